# revision 18
# baseline (speedup 1.0000x reference)
"""DenseFiLMResBlock Trainium2 kernel (software-pipelined, bf16 matmul path).

Shape: B=32, S=1024, D=1024, E=128. Data-parallel over batch: 8 cores x 4
samples. Feature-major on-device layout ([D partition-blocks, S free]); the
host pre-transposes x per core (and casts to bf16) and post-transposes the
fp32 output.

Key differences vs the fp32r baseline:
  * Matmul path in bf16 (weights + activations): FWL weight loads, half the
    SBUF/DMA footprint. PSUM accumulation stays fp32; rel-err ~2e-3 << 2e-2.
  * LN rsqrt via bit-trick + 2 Newton steps on the vector engine: the scalar
    engine never leaves the silu_and_others activation table set, killing
    ~1.3us table reloads per LayerNorm chain.
  * Software-pipelined PE schedule: block order M1(0) M1(1) M2(0) M1(2)
    M2(1) M1(3) M2(2) M2(3); every LN stats chain + eltwise block executes
    under the previous/next sample's 27us matmul block, so the PE stream has
    no structural bubbles (also keeps the PE HAM clock-gate warm).
  * LN chain cross-partition reduce/broadcast matmuls are injected between
    accumulation groups of the FOLLOWING mm block, after their inputs are
    ready, so the in-order PE never stalls on them.
  * Residual reads the resident bf16 xT tile: no second x load from DRAM.
"""
import numpy as np

import concourse.bacc as bacc
import concourse.tile as tile
from concourse import mybir
from concourse import bass2jax

B, S, D, E = 32, 1024, 1024, 128
N_CORES = 8
BL = B // N_CORES          # samples per core
KB = D // 128              # 8 d-blocks
P = 128
SH = S // 2                # 512: free-dim half (one PSUM bank per group)
F32 = mybir.dt.float32
BF16 = mybir.dt.bfloat16
I32 = mybir.dt.int32
AF = mybir.ActivationFunctionType
ALU = mybir.AluOpType

TWO_PI = 2.0 * np.pi
INV_2PI = float(1.0 / TWO_PI)
C1 = 6.28125                       # exact in fp32
C2 = float(TWO_PI - 6.28125)
MAGIC = 12582912.0                 # 1.5*2^23: fp32 round-to-nearest-int trick
HALF_PI = float(np.pi / 2)
EPS = 1e-5
RSQRT_MAGIC = 0x5F3759DF           # quake rsqrt seed

_BUILD_CACHE = {}
_TRACE_SIM = False   # set True to publish a cost-model scheduling trace
_REPEAT = 1          # >1: repeat the whole per-sample pipeline (timing only)


def _build(with_affine: bool, repeat: int = 1):
    nc = bacc.Bacc("TRN2", target_bir_lowering=False, debug=False,
                   num_devices=N_CORES)

    xT_d = nc.dram_tensor("xT", [BL, D, S], BF16, kind="ExternalInput")
    t_d = nc.dram_tensor("t", [BL], F32, kind="ExternalInput")
    freqs_d = nc.dram_tensor("freqs", [E // 2], F32, kind="ExternalInput")
    W1_d = nc.dram_tensor("W1", [E, 4 * E], BF16, kind="ExternalInput")
    b1_d = nc.dram_tensor("b1", [4 * E], F32, kind="ExternalInput")
    W2_d = nc.dram_tensor("W2", [4 * E, 4 * E], BF16, kind="ExternalInput")
    b2_d = nc.dram_tensor("b2", [4 * E], F32, kind="ExternalInput")
    Wsc_d = nc.dram_tensor("Wsc", [4 * E, D], BF16, kind="ExternalInput")
    bsc_d = nc.dram_tensor("bsc", [D], F32, kind="ExternalInput")
    Wsh_d = nc.dram_tensor("Wsh", [4 * E, D], BF16, kind="ExternalInput")
    bsh_d = nc.dram_tensor("bsh", [D], F32, kind="ExternalInput")
    Win_d = nc.dram_tensor("Win", [D, D], BF16, kind="ExternalInput")
    bin_d = nc.dram_tensor("bin", [D], F32, kind="ExternalInput")
    Wout_d = nc.dram_tensor("Wout", [D, D], BF16, kind="ExternalInput")
    bout_d = nc.dram_tensor("bout", [D], F32, kind="ExternalInput")
    if with_affine:
        gT_d = nc.dram_tensor("gammaT", [D, S], BF16, kind="ExternalInput")
        bT_d = nc.dram_tensor("betaT", [D, S], BF16, kind="ExternalInput")
    outT_d = nc.dram_tensor("outT", [BL, D, S], F32, kind="ExternalOutput")

    with tile.TileContext(nc, trace_sim=_TRACE_SIM) as tc:
        nbig = 2 if with_affine else 3
        with tc.tile_pool(name="consts", bufs=1) as consts, \
             tc.tile_pool(name="wts", bufs=1) as wts, \
             tc.tile_pool(name="filmw", bufs=1) as filmw, \
             tc.tile_pool(name="small", bufs=2) as small, \
             tc.tile_pool(name="film_sm", bufs=4) as film_sm, \
             tc.tile_pool(name="bigx", bufs=3) as bigx, \
             tc.tile_pool(name="bigu", bufs=nbig) as bigu, \
             tc.tile_pool(name="bigy", bufs=2) as bigy, \
             tc.tile_pool(name="stream", bufs=4) as stream, \
             tc.tile_pool(name="psum_mm", bufs=3, space="PSUM") as psum_mm, \
             tc.tile_pool(name="psum_sm", bufs=2, space="PSUM") as psum_sm:

            # ---------- constants ----------
            ones_k = consts.tile([P, 1], F32)
            nc.vector.memset(ones_k, 1.0 / (KB * P))   # stats sums -> means
            ones_m = consts.tile([1, P], F32)
            nc.vector.memset(ones_m, 1.0)
            magic_t = consts.tile([1, 1], I32)
            nc.vector.memset(magic_t, RSQRT_MAGIC)
            one_i = consts.tile([1, 1], I32)
            nc.vector.memset(one_i, 1)
            halfpi_t = consts.tile([E // 2, 1], F32)
            nc.vector.memset(halfpi_t, HALF_PI)

            def load_bias_T(dram, nblk, name):
                t_ = consts.tile([P, nblk], F32, tag=name)
                nc.sync.dma_start(
                    out=t_, in_=dram.ap().rearrange("(a p) -> p a", p=P))
                return t_

            b1T = load_bias_T(b1_d, 4, "b1T")
            b2T = load_bias_T(b2_d, 4, "b2T")
            bscT = load_bias_T(bsc_d, KB, "bscT")
            bshT = load_bias_T(bsh_d, KB, "bshT")
            binT = load_bias_T(bin_d, KB, "binT")
            boutT = load_bias_T(bout_d, KB, "boutT")

            scaleT = consts.tile([P, KB, BL], F32, tag="scaleT")
            shiftT = consts.tile([P, KB, BL], F32, tag="shiftT")

            # ---------- FiLM (prologue; excluded from the repeat loop) ----
            # noise encoding, feature-major embT [64, BL]
            t_bc = film_sm.tile([E // 2, BL], F32, tag="film_sm")
            nc.sync.dma_start(
                out=t_bc, in_=t_d.ap()[None, :].to_broadcast((E // 2, BL)))
            fr = film_sm.tile([E // 2, 1], F32, tag="film_sm")
            nc.sync.dma_start(out=fr, in_=freqs_d.ap()[:, None])

            W1_sb = filmw.tile([P, 4 * E], BF16, tag="fW1")
            nc.sync.dma_start(out=W1_sb, in_=W1_d.ap())
            W2_sb = filmw.tile([P, 4, 4 * E], BF16, tag="fW2")
            for kb in range(4):
                nc.sync.dma_start(out=W2_sb[:, kb, :],
                                  in_=W2_d.ap()[kb * P:(kb + 1) * P, :])
            Wsc_sb = filmw.tile([P, 4, D], BF16, tag="fWsc")
            Wsh_sb = filmw.tile([P, 4, D], BF16, tag="fWsh")
            for kb in range(4):
                nc.sync.dma_start(out=Wsc_sb[:, kb, :],
                                  in_=Wsc_d.ap()[kb * P:(kb + 1) * P, :])
                nc.sync.dma_start(out=Wsh_sb[:, kb, :],
                                  in_=Wsh_d.ap()[kb * P:(kb + 1) * P, :])

            emb = film_sm.tile([E // 2, BL], F32, tag="film_sm")
            nc.vector.tensor_scalar(out=emb, in0=t_bc, scalar1=5000.0,
                                    scalar2=fr, op0=ALU.mult, op1=ALU.mult)
            # Cody-Waite: k = round(emb/2pi); er = (emb - k*C1) - k*C2
            r_ = film_sm.tile([E // 2, BL], F32, tag="film_sm")
            nc.vector.tensor_scalar(out=r_, in0=emb, scalar1=INV_2PI,
                                    scalar2=MAGIC, op0=ALU.mult, op1=ALU.add)
            k_ = film_sm.tile([E // 2, BL], F32, tag="film_sm")
            nc.vector.tensor_scalar(out=k_, in0=r_, scalar1=MAGIC,
                                    scalar2=None, op0=ALU.subtract)
            kc1 = film_sm.tile([E // 2, BL], F32, tag="film_sm")
            nc.vector.tensor_scalar(out=kc1, in0=k_, scalar1=C1,
                                    scalar2=None, op0=ALU.mult)
            er = film_sm.tile([E // 2, BL], F32, tag="film_sm")
            nc.vector.tensor_tensor(out=er, in0=emb, in1=kc1,
                                    op=ALU.subtract)
            kc2 = film_sm.tile([E // 2, BL], F32, tag="film_sm")
            nc.vector.tensor_scalar(out=kc2, in0=k_, scalar1=C2,
                                    scalar2=None, op0=ALU.mult)
            er2 = film_sm.tile([E // 2, BL], F32, tag="film_sm")
            nc.vector.tensor_tensor(out=er2, in0=er, in1=kc2,
                                    op=ALU.subtract)   # in [-pi, pi]
            hT = film_sm.tile([E, BL], BF16, tag="hT")
            nc.scalar.activation(out=hT[0:E // 2, :], in_=er2, func=AF.Sin)
            # cos(y) = sin(pi/2 - |y|)  (cos even; keeps |arg| <= pi/2)
            neg = film_sm.tile([E // 2, BL], F32, tag="film_sm")
            nc.vector.tensor_scalar(out=neg, in0=er2, scalar1=-1.0,
                                    scalar2=None, op0=ALU.mult)
            ab = film_sm.tile([E // 2, BL], F32, tag="film_sm")
            nc.vector.tensor_tensor(out=ab, in0=er2, in1=neg, op=ALU.max)
            nc.scalar.activation(out=hT[E // 2:E, :], in_=ab, func=AF.Sin,
                                 scale=-1.0, bias=halfpi_t)

            # h1 = silu(W1.T @ hT + b1): [512, BL] as [128, 4, BL]
            h1 = film_sm.tile([P, 4, BL], BF16, tag="h1")
            for mb in range(4):
                ps = psum_sm.tile([P, BL], F32, tag="sm")
                nc.tensor.matmul(ps, W1_sb[:, mb * P:(mb + 1) * P], hT,
                                 start=True, stop=True)
                nc.scalar.activation(out=h1[:, mb, :], in_=ps, func=AF.Silu,
                                     bias=b1T[:, mb:mb + 1])
            # h2 = W2.T @ h1 + b2
            h2 = film_sm.tile([P, 4, BL], BF16, tag="h2")
            for mb in range(4):
                ps = psum_sm.tile([P, BL], F32, tag="sm")
                for kb in range(4):
                    nc.tensor.matmul(ps, W2_sb[:, kb, mb * P:(mb + 1) * P],
                                     h1[:, kb, :], start=(kb == 0),
                                     stop=(kb == 3))
                nc.scalar.activation(out=h2[:, mb, :], in_=ps,
                                     func=AF.Identity, bias=b2T[:, mb:mb + 1])
            # scaleT = Wsc.T @ h2 + bsc ; shiftT = Wsh.T @ h2 + bsh
            for mb in range(KB):
                ps = psum_sm.tile([P, BL], F32, tag="sm")
                for kb in range(4):
                    nc.tensor.matmul(ps, Wsc_sb[:, kb, mb * P:(mb + 1) * P],
                                     h2[:, kb, :], start=(kb == 0),
                                     stop=(kb == 3))
                nc.scalar.activation(out=scaleT[:, mb, :], in_=ps,
                                     func=AF.Identity, bias=bscT[:, mb:mb + 1])
                ps2 = psum_sm.tile([P, BL], F32, tag="sm")
                for kb in range(4):
                    nc.tensor.matmul(ps2, Wsh_sb[:, kb, mb * P:(mb + 1) * P],
                                     h2[:, kb, :], start=(kb == 0),
                                     stop=(kb == 3))
                nc.scalar.activation(out=shiftT[:, mb, :], in_=ps2,
                                     func=AF.Identity, bias=bshT[:, mb:mb + 1])

            # big weights: Win first (needed by M1(0)), Wout later (M2(0))
            Win_sb = wts.tile([P, KB, D], BF16, tag="Win")
            Wout_sb = wts.tile([P, KB, D], BF16, tag="Wout")
            for kb in range(KB):
                nc.sync.dma_start(out=Win_sb[:, kb, :],
                                  in_=Win_d.ap()[kb * P:(kb + 1) * P, :])

            # ---------- per-sample pipeline helpers ----------
            class ChainLN:
                """Cross-partition LN stats -> per-partition (rs, -mu*rs)
                broadcast + effective scale/bias vectors. All serial math on
                DVE (Newton rsqrt; no scalar-engine table switches); the two
                PE touches (reduce, broadcast) are emitted via stage2/stage3
                so callers can inject them between mm accumulation groups."""

                def __init__(self, mv, b):
                    self.mv = mv
                    self.b = b
                    st_t = small.tile([P, 2 * KB], F32, tag="ch_st")
                    nc.vector.tensor_scalar(out=st_t[:, 0:KB], in0=mv[:, :, 0],
                                            scalar1=1.0, scalar2=None,
                                            op0=ALU.mult)
                    sq = small.tile([P, KB], F32, tag="ch_sq")
                    nc.vector.tensor_tensor(out=sq, in0=mv[:, :, 0],
                                            in1=mv[:, :, 0], op=ALU.mult)
                    nc.vector.tensor_tensor(out=st_t[:, KB:2 * KB], in0=sq,
                                            in1=mv[:, :, 1], op=ALU.add)
                    self.st_t = st_t

                def stage2(self):
                    # PE: column sums over partitions -> [1, 2*KB]
                    self.ps_s = psum_sm.tile([1, 2 * KB], F32, tag="sm")
                    nc.tensor.matmul(self.ps_s, ones_k, self.st_t,
                                     start=True, stop=True)

                def stage3(self):
                    # DVE: E[x], E[x^2] -> v=var+eps -> Newton rsqrt -> rsnm
                    red = small.tile([1, 2], F32, tag="ch_red2")
                    nc.vector.reduce_sum(red[:, 0:1], self.ps_s[:, 0:KB],
                                         axis=mybir.AxisListType.X)
                    nc.vector.reduce_sum(red[:, 1:2], self.ps_s[:, KB:2 * KB],
                                         axis=mybir.AxisListType.X)
                    v = small.tile([1, 1], F32, tag="ch_v")
                    t1 = small.tile([1, 1], F32, tag="ch_t1")
                    nc.vector.tensor_scalar(out=t1, in0=red[:, 0:1],
                                            scalar1=red[:, 0:1], scalar2=-1.0,
                                            op0=ALU.mult, op1=ALU.mult)
                    nc.vector.tensor_scalar(out=v, in0=red[:, 1:2],
                                            scalar1=t1, scalar2=EPS,
                                            op0=ALU.add, op1=ALU.add)
                    # y0 = bitcast(MAGIC - (bitcast(v) >> 1))
                    rsnm = small.tile([1, 2], F32, tag="ch_rsnm")
                    y = rsnm[:, 0:1]
                    hi = small.tile([1, 1], I32, tag="ch_hi")
                    nc.vector.tensor_tensor(out=hi, in0=v.bitcast(I32),
                                            in1=one_i,
                                            op=ALU.logical_shift_right)
                    nc.vector.tensor_tensor(out=y.bitcast(I32), in0=magic_t,
                                            in1=hi, op=ALU.subtract)
                    vh = small.tile([1, 1], F32, tag="ch_vh")
                    nc.vector.tensor_scalar(out=vh, in0=v, scalar1=-0.5,
                                            scalar2=None, op0=ALU.mult)
                    tn = small.tile([1, 1], F32, tag="ch_tn")
                    for _ in range(2):   # 2 Newton steps: rel err ~5e-6
                        nc.vector.tensor_tensor(out=tn, in0=y, in1=y,
                                                op=ALU.mult)
                        nc.vector.tensor_scalar(out=tn, in0=tn, scalar1=vh,
                                                scalar2=1.5, op0=ALU.mult,
                                                op1=ALU.add)
                        nc.vector.tensor_tensor(out=y, in0=y, in1=tn,
                                                op=ALU.mult)
                    # rsnm[1] = -mean * rs
                    nc.vector.tensor_scalar(out=rsnm[:, 1:2], in0=y,
                                            scalar1=red[:, 0:1], scalar2=-1.0,
                                            op0=ALU.mult, op1=ALU.mult)
                    # PE broadcast across partitions
                    bc = psum_sm.tile([P, 2], F32, tag="sm")
                    nc.tensor.matmul(bc, ones_m, rsnm, start=True, stop=True)
                    self.bc = bc
                    if with_affine:
                        # ACT scale/bias operands must live in SBUF
                        bc_sb = small.tile([P, 2], F32, tag="ch_bcs")
                        nc.vector.tensor_scalar(out=bc_sb, in0=bc,
                                                scalar1=1.0, scalar2=None,
                                                op0=ALU.mult)
                        self.bc_sb = bc_sb
                    # effective per-partition scale/bias for the fused ACT
                    b = self.b
                    seff = small.tile([P, KB], F32, tag="seff")
                    nc.vector.tensor_tensor(out=seff, in0=scaleT[:, :, b],
                                            in1=bc[:, 0:1].to_broadcast((P, KB)),
                                            op=ALU.mult)
                    beff = small.tile([P, KB], F32, tag="beff")
                    nc.vector.tensor_tensor(out=beff, in0=scaleT[:, :, b],
                                            in1=bc[:, 1:2].to_broadcast((P, KB)),
                                            op=ALU.mult)
                    nc.vector.tensor_tensor(out=beff, in0=beff,
                                            in1=shiftT[:, :, b], op=ALU.add)
                    self.seff, self.beff = seff, beff

            def emit_L(b):
                """Load xT(b) + LN1 per-partition stats."""
                xt = bigx.tile([P, KB, 2, SH], BF16, tag="x")
                mv1 = small.tile([P, KB, 2], F32, tag="mv1")
                st1 = small.tile([P, KB, 2, 6], F32, tag="bnst1")
                for kb in range(KB):
                    nc.sync.dma_start(
                        out=xt[:, kb],
                        in_=xT_d.ap()[b, kb * P:(kb + 1) * P, :]
                        .rearrange("p (a q) -> p a q", a=2))
                    nc.vector.bn_stats(out=st1[:, kb, 0, :],
                                       in_=xt[:, kb, 0, :])
                    nc.vector.bn_stats(out=st1[:, kb, 1, :],
                                       in_=xt[:, kb, 1, :])
                    nc.vector.bn_aggr(out=mv1[:, kb, :], in_=st1[:, kb, :, :])
                return xt, mv1

            def emit_A(src, ch, b):
                """u = Silu(seff*src + beff): one full-row ACT per kb."""
                u = bigu.tile([P, KB, 2, SH], BF16, tag="u")
                if not with_affine:
                    for kb in range(KB):
                        nc.scalar.activation(out=u[:, kb],
                                             in_=src[:, kb],
                                             func=AF.Silu,
                                             scale=ch.seff[:, kb:kb + 1],
                                             bias=ch.beff[:, kb:kb + 1])
                    return u
                # general affine: n = gamma*(x*rs+nmr)+beta, u = Silu(sc*n+sh)
                for st in range(2):
                    sl = slice(st * SH, (st + 1) * SH)
                    for kb in range(KB):
                        gt = stream.tile([P, SH], BF16, tag="gT")
                        bt = stream.tile([P, SH], BF16, tag="bT")
                        nc.sync.dma_start(
                            out=gt, in_=gT_d.ap()[kb * P:(kb + 1) * P, sl])
                        nc.sync.dma_start(
                            out=bt, in_=bT_d.ap()[kb * P:(kb + 1) * P, sl])
                        n_ = stream.tile([P, SH], F32, tag="n_")
                        nc.scalar.activation(out=n_, in_=src[:, kb, st, :],
                                             func=AF.Identity,
                                             scale=ch.bc_sb[:, 0:1],
                                             bias=ch.bc_sb[:, 1:2])
                        nc.vector.tensor_tensor(out=n_, in0=n_, in1=gt,
                                                op=ALU.mult)
                        nc.vector.tensor_tensor(out=n_, in0=n_, in1=bt,
                                                op=ALU.add)
                        nc.scalar.activation(out=u[:, kb, st, :], in_=n_,
                                             func=AF.Silu,
                                             scale=scaleT[:, kb, b:b + 1],
                                             bias=shiftT[:, kb, b:b + 1])
                return u

            def mm_block(W_sb, u, evict_fn, inject):
                """8 pairs (mb-major) of 16 accumulating matmuls into a
                2-bank PSUM pair-tile; evict_fn(mb, ps) emits the non-PE
                drain once per mb row; inject maps pair-index -> closures
                emitted right after that pair (chain PE ops)."""
                for mb in range(KB):
                    ps = psum_mm.tile([P, 2, SH], F32, tag="mmps")
                    for st in range(2):
                        for kb in range(KB):
                            nc.tensor.matmul(
                                ps[:, st, :], W_sb[:, kb, mb * P:(mb + 1) * P],
                                u[:, kb, st, :],
                                start=(kb == 0), stop=(kb == KB - 1))
                    evict_fn(mb, ps)
                    for fn in inject.get(mb + 1, ()):
                        fn()

            # ---------- software-pipelined schedule ----------
            # Block sequence per iteration: M1(0) M1(1) M2(0) M1(2) M2(1)
            # M1(3) M2(2) M2(3); concatenated across `repeat` iterations.
            seq = []
            for r in range(repeat):
                for b in range(BL):
                    seq.append(("M1", r, b))
                    if b > 0:
                        seq.append(("M2", r, b - 1))
                seq.append(("M2", r, BL - 1))
            m1_index = {}
            for i, (kind, r, b) in enumerate(seq):
                if kind == "M1":
                    m1_index[len(m1_index)] = i
            sample_of_m1 = {v: k for k, v in m1_index.items()}  # blockidx->s

            state = {}      # per-sample live tiles: s -> dict
            pending = []    # [(chain, post_fn)] to inject into next block

            def prep_sample(s):
                r, b = divmod(s, BL)
                xt, mv1 = emit_L(b)
                ch = ChainLN(mv1, b)
                st_ = {"xt": xt, "ch1": ch}
                state[s] = st_

                def post():
                    st_["u1"] = emit_A(xt, ch, b)
                return ch, post

            # prologue: samples 0 and 1
            ch0, post0 = prep_sample(0)
            ch0.stage2()
            ch0.stage3()
            post0()
            if len(m1_index) > 1:
                ch1, post1 = prep_sample(1)
                pending.append((ch1, post1))

            first_m2 = True
            for k, (kind, r, b) in enumerate(seq):
                s = r * BL + b
                # prefetch the sample whose M1 sits two blocks ahead
                nxt = sample_of_m1.get(k + 2)
                if nxt is not None and nxt >= 2:
                    chn, postn = prep_sample(nxt)
                    pending.append((chn, postn))

                cur, pending = pending, []
                inject = {}
                slots = [(1, 3), (4, 6)]
                for (chx, _), (g2, g3) in zip(cur, slots):
                    inject.setdefault(g2, []).append(chx.stage2)
                    inject.setdefault(g3, []).append(chx.stage3)

                st_ = state[s]
                if kind == "M1":
                    y1 = bigy.tile([P, KB, 2, SH], BF16, tag="y")
                    mv2 = small.tile([P, KB, 2], F32, tag="mv2")
                    st2 = small.tile([P, KB, 2, 6], F32, tag="bnst2")

                    def ev1(mb, ps, y1=y1, mv2=mv2, st2=st2):
                        # bias-evict full row on ScalarE, per-half stats DVE
                        nc.scalar.activation(out=y1[:, mb], in_=ps,
                                             func=AF.Identity,
                                             bias=binT[:, mb:mb + 1])
                        nc.vector.bn_stats(out=st2[:, mb, 0, :],
                                           in_=y1[:, mb, 0, :])
                        nc.vector.bn_stats(out=st2[:, mb, 1, :],
                                           in_=y1[:, mb, 1, :])
                        nc.vector.bn_aggr(out=mv2[:, mb, :],
                                          in_=st2[:, mb, :, :])

                    mm_block(Win_sb, st_["u1"], ev1, inject)
                    st_["y1"] = y1
                    ch2 = ChainLN(mv2, b)
                    st_["ch2"] = ch2

                    def post2(st_=st_, ch2=ch2, b=b):
                        st_["u2"] = emit_A(st_["y1"], ch2, b)
                    pending.append((ch2, post2))
                else:
                    if first_m2:
                        first_m2 = False
                        for kb in range(KB):
                            nc.sync.dma_start(
                                out=Wout_sb[:, kb, :],
                                in_=Wout_d.ap()[kb * P:(kb + 1) * P, :])
                    xt = st_["xt"]

                    def ev2(mb, ps, xt=xt, b=b):
                        # PSUM evict (+bout) on ScalarE frees the bank even
                        # when DVE is busy with prefetched bn_stats; the
                        # residual add runs SBUF-side on DVE off the WAR path
                        ot = stream.tile([P, 2, SH], F32, tag="ot")
                        nc.scalar.activation(out=ot, in_=ps, func=AF.Identity,
                                             bias=boutT[:, mb:mb + 1])
                        nc.vector.tensor_tensor(out=ot, in0=ot,
                                                in1=xt[:, mb], op=ALU.add)
                        nc.sync.dma_start(
                            out=outT_d.ap()[b, mb * P:(mb + 1) * P, :]
                            .rearrange("p (a q) -> p a q", a=2),
                            in_=ot)

                    mm_block(Wout_sb, st_["u2"], ev2, inject)
                    del state[s]

                for _, post in cur:
                    post()

    nc.finalize()
    return nc


def _get_nc(with_affine: bool, repeat: int = 1):
    key = (with_affine, repeat)
    if key not in _BUILD_CACHE:
        _BUILD_CACHE[key] = _build(with_affine, repeat)
    return _BUILD_CACHE[key]


_RUNNER_CACHE = {}


def _get_runner(nc):
    """Replicates bass2jax.run_bass_via_pjrt but jits ONCE per nc so repeat
    calls skip re-trace/re-lower (the NEFF itself is cached by neuronxcc)."""
    key = id(nc)
    if key in _RUNNER_CACHE:
        return _RUNNER_CACHE[key]
    import jax

    from jax.experimental.shard_map import shard_map
    from jax.sharding import Mesh, PartitionSpec

    try:
        jax.config.update("jax_compilation_cache_dir", "/tmp/jax_comp_cache")
        jax.config.update("jax_persistent_cache_min_compile_time_secs", 2.0)
    except Exception:
        pass
    bass2jax.install_neuronx_cc_hook()
    partition_name = (nc.partition_id_tensor.name
                      if nc.partition_id_tensor else None)
    in_names, out_names, out_avals, zero_outs = [], [], [], []
    for alloc in nc.m.functions[0].allocations:
        if not isinstance(alloc, mybir.MemoryLocationSet):
            continue
        name = alloc.memorylocations[0].name
        if alloc.kind == "ExternalInput":
            if name != partition_name:
                in_names.append(name)
        elif alloc.kind == "ExternalOutput":
            shape = tuple(alloc.tensor_shape)
            dtype = mybir.dt.np(alloc.dtype)
            out_names.append(name)
            out_avals.append(jax.core.ShapedArray(shape, dtype))
            zero_outs.append(np.zeros(shape, dtype))
    n_params = len(in_names)
    all_in_names = list(in_names) + list(out_names)
    if partition_name is not None:
        all_in_names.append(partition_name)
    donate = tuple(range(n_params, n_params + len(out_names)))

    def _body(*args):
        operands = list(args)
        if partition_name is not None:
            operands.append(bass2jax.partition_id_tensor())
        outs = bass2jax._bass_exec_p.bind(
            *operands,
            out_avals=tuple(out_avals),
            in_names=tuple(all_in_names),
            out_names=tuple(out_names),
            lowering_input_output_aliases=(),
            sim_require_finite=True,
            sim_require_nnan=True,
            nc=nc,
        )
        return tuple(outs)

    devices = jax.devices()[:N_CORES]
    mesh = Mesh(np.asarray(devices), ("core",))
    n_out = len(out_names)
    sharded = jax.jit(
        shard_map(_body, mesh=mesh,
                  in_specs=(PartitionSpec("core"),) * (n_params + n_out),
                  out_specs=(PartitionSpec("core"),) * n_out,
                  check_rep=False),
        donate_argnums=donate, keep_unused=True)
    runner = {
        "sharded": sharded, "in_names": in_names, "out_names": out_names,
        "out_avals": out_avals, "zero_outs": zero_outs, "mesh": mesh,
    }
    _RUNNER_CACHE[key] = runner
    return runner


def _fingerprint(a):
    b = np.ascontiguousarray(a).reshape(-1).view(np.uint8)
    step = max(1, b.size // 8192)
    return (a.shape, a.dtype.str, hash(b[::step][:8192].tobytes()))


def _run_full(nc, full_map, static_names=()):
    """Run the SPMD program on concatenated-along-axis-0 inputs.

    static_names: inputs cached device-side by content fingerprint (weights).
    Output buffers are donated; since the kernel overwrites every element of
    outT, the previous call's outputs are recycled as the donated buffers.
    """
    import jax
    from jax.sharding import NamedSharding, PartitionSpec

    r = _get_runner(nc)
    sh = NamedSharding(r["mesh"], PartitionSpec("core"))
    cache = r.setdefault("dev_cache", {})
    args = []
    for name in r["in_names"]:
        a = np.asarray(full_map[name])
        if name in static_names:
            fp = _fingerprint(a)
            hit = cache.get(name)
            if hit is None or hit[0] != fp:
                cache[name] = (fp, jax.device_put(a, sh))
            args.append(cache[name][1])
        else:
            args.append(jax.device_put(a, sh))
    donate = r.get("donate_next")
    if donate is None:
        donate = [jax.device_put(
            np.zeros((N_CORES * z.shape[0], *z.shape[1:]), z.dtype), sh)
            for z in r["zero_outs"]]
    out_arrs = r["sharded"](*args, *donate)
    outs = {name: np.asarray(out_arrs[i])
            for i, name in enumerate(r["out_names"])}
    r["donate_next"] = list(out_arrs)
    return outs


_FREQS = np.exp(
    np.arange(E // 2, dtype=np.float32) * (-np.log(10000.0) / (E // 2 - 1))
).astype(np.float32)


def _bf16_dtype():
    import ml_dtypes
    return np.dtype(ml_dtypes.bfloat16)


def _prep_weights(W1, b1, W2, b2, Wsc, bsc, Wsh, bsh, W_in, b_in, W_out,
                  b_out, gamma=None, beta=None):
    bf = _bf16_dtype()
    weights = {
        "W1": np.ascontiguousarray(W1, dtype=np.float32).astype(bf),
        "b1": np.ascontiguousarray(b1, dtype=np.float32),
        "W2": np.ascontiguousarray(W2, dtype=np.float32).astype(bf),
        "b2": np.ascontiguousarray(b2, dtype=np.float32),
        "Wsc": np.ascontiguousarray(Wsc, dtype=np.float32).astype(bf),
        "bsc": np.ascontiguousarray(bsc, dtype=np.float32),
        "Wsh": np.ascontiguousarray(Wsh, dtype=np.float32).astype(bf),
        "bsh": np.ascontiguousarray(bsh, dtype=np.float32),
        "Win": np.ascontiguousarray(W_in, dtype=np.float32).astype(bf),
        "bin": np.ascontiguousarray(b_in, dtype=np.float32),
        "Wout": np.ascontiguousarray(W_out, dtype=np.float32).astype(bf),
        "bout": np.ascontiguousarray(b_out, dtype=np.float32),
    }
    if gamma is not None:
        weights["gammaT"] = np.ascontiguousarray(
            np.asarray(gamma, dtype=np.float32).T).astype(bf)
        weights["betaT"] = np.ascontiguousarray(
            np.asarray(beta, dtype=np.float32).T).astype(bf)
    return weights


def kernel(x, t, W1, b1, W2, b2, Wsc, bsc, Wsh, bsh, gamma, beta,
           W_in, b_in, W_out, b_out):
    x = np.asarray(x, dtype=np.float32)
    t = np.asarray(t, dtype=np.float32)
    gamma = np.asarray(gamma, dtype=np.float32)
    beta = np.asarray(beta, dtype=np.float32)
    with_affine = not (np.all(gamma == 1.0) and np.all(beta == 0.0))
    bf = _bf16_dtype()

    weights = _prep_weights(
        W1, b1, W2, b2, Wsc, bsc, Wsh, bsh, W_in, b_in, W_out, b_out,
        gamma if with_affine else None, beta if with_affine else None)

    nc = _get_nc(with_affine)
    # concat-along-axis-0 == per-core shards stacked: one transpose, no split
    full_map = {
        "xT": np.ascontiguousarray(x.transpose(0, 2, 1)).astype(bf),
        "t": np.ascontiguousarray(t),
        "freqs": np.tile(_FREQS, N_CORES),
    }
    static = []
    for name, w in weights.items():
        full_map[name] = np.concatenate([w] * N_CORES, axis=0)
        static.append(name)
    outs = _run_full(nc, full_map, static_names=tuple(static))
    outT = outs["outT"].reshape(B, D, S)
    return np.ascontiguousarray(outT.transpose(0, 2, 1))   # [B, S, D]
